# revision 1
# baseline (speedup 1.0000x reference)
"""Trainium2 Bass kernel for nn_ContrastiveCriterion.

Reference semantics (per sample b of B=2, N=4096, D=512):
    refer = l2_normalize(emb_point[b][pos_idx[b]])      # [N, D]
    key   = l2_normalize(emb_text[b])                   # [N, D]
    sim   = refer @ key.T                               # [N, N]
    ce_p[i] = logsumexp_j(ls*sim[i,j]) - ls*sim[i,i]
    ce_t[j] = logsumexp_i(ls*sim[i,j]) - ls*sim[j,j]
    loss_b  = mean_i(0.5*(ce_p+ce_t)*dist_norm[b])
    rank_b  = sum_ij relu(sim[i,j] - sim[j,j])
    out = (mean_b loss_b, 0.5 * mean_b rank_b)

Sharding: 8 cores = 2 samples x 4 row-chunks of 1024. Core (b, q) computes
rows [q*1024, q*1024+1024) of both sim (phase A -> exp rowsums + diag) and
simT (phase B -> exp rowsums = sim colsums, and relu rank rowsums). The host
rolls each core's refer/key arrays by -q*1024 so one SPMD program (chunk at
rows 0..1024, diagonal at free offset ti*128) serves all cores. L2 norms are
computed on device (squares -> ones-matmul partition-reduce broadcast ->
sqrt -> reciprocal -> scale). Host does the gather, bf16 cast, and the final
O(N) f64 reductions over the per-row stats each core returns.
"""

import numpy as np
import ml_dtypes

import concourse.bass as bass
import concourse.tile as tile
import concourse.mybir as mybir
from concourse.bass_utils import run_bass_kernel_spmd

B, N, D = 2, 4096, 512
P = 128                 # SBUF partitions
KC = D // P             # 4 contraction chunks
QPER = 4                # cores per sample
CHUNK = N // QPER       # 1024 rows per core
TI = CHUNK // P         # 8 output tiles per core
HALF = 2048             # psum tile free width (4 banks)
JB4 = HALF // 512       # 512-wide matmul blocks per psum tile

bf16 = mybir.dt.bfloat16
f32 = mybir.dt.float32

# set by kernel() for test harness introspection
LAST_RESULT = None

# walrus codegen for TRN2 CTRL instructions (Drain) accepts a limited number
# of sync-wait slots; Tile's kernel-tail drain can carry one wait per live
# semaphore.  Split any over-limit drain into a chain of drains, each
# carrying at most MAX_DRAIN_WAITS waits (same-engine program order makes
# the chain equivalent to the single multi-wait drain).
MAX_DRAIN_WAITS = 1


def _split_drain_waits(nc: bass.Bass, max_waits: int = MAX_DRAIN_WAITS) -> None:
    for fn in nc.m.functions:
        for bb in fn.blocks:
            insts = list(bb.instructions)
            out, n_extra = [], 0
            for ins in insts:
                si = ins.sync_info
                if si is not None and si.on_wait and len(si.on_wait) > max_waits:
                    waits = list(si.on_wait)
                    for k in range(0, len(waits) - max_waits, max_waits):
                        extra = mybir.InstDrain(
                            name=f"{ins.name}_prewait{k}",
                            ins=[],
                            outs=[],
                        )
                        extra.engine = ins.engine
                        extra.sync_info = mybir.SyncInfo(
                            on_wait=waits[k: k + max_waits], on_update=[]
                        )
                        out.append(extra)
                        n_extra += 1
                    si.on_wait = waits[len(waits) - max_waits:]
                out.append(ins)
            if n_extra:
                bb.instructions[:] = out


def build_program(logit_scale: float, split_loads: bool = False,
                  relu_split: bool = False) -> bass.Bass:
    nc = bass.Bass()

    pt = nc.declare_dram_parameter("pt", [N, D], bf16, isOutput=False)
    tx = nc.declare_dram_parameter("tx", [N, D], bf16, isOutput=False)
    out_sp = nc.declare_dram_parameter("out_sp", [P, 2 * TI], f32, isOutput=True)
    out_st = nc.declare_dram_parameter("out_st", [P, 2 * TI], f32, isOutput=True)
    out_r = nc.declare_dram_parameter("out_r", [P, 2 * TI], f32, isOutput=True)
    out_d = nc.declare_dram_parameter("out_d", [P, TI], f32, isOutput=True)

    ident_dram = nc.inline_tensor(np.eye(P, dtype=np.float32), name="ident")

    Act = mybir.ActivationFunctionType
    Alu = mybir.AluOpType

    with tile.TileContext(nc) as tc:
        with tc.tile_pool(name="main", bufs=1) as pmain:
            # persistent transposed operands: xT[c][d_local, row]
            rT = [pmain.tile([P, N], bf16, name=f"rT{c}", tag=f"rT{c}") for c in range(KC)]
            kT = [pmain.tile([P, N], bf16, name=f"kT{c}", tag=f"kT{c}") for c in range(KC)]
            # normalized copy of refer (phase B moving operand); refer stays
            # raw for phase A's lhsT (row norm rides on the ACT exp scale)
            rTn = [pmain.tile([P, N], bf16, name=f"rTn{c}", tag=f"rTn{c}") for c in range(KC)]
            ones_t = pmain.tile([P, P], bf16, name="ones_t", tag="ones_t")
            ident_sb = pmain.tile([P, P], f32, name="ident_sb", tag="ident_sb")
            sp_parts = pmain.tile([P, 2 * TI], f32, name="sp_parts", tag="sp_parts")
            st_parts = pmain.tile([P, 2 * TI], f32, name="st_parts", tag="st_parts")
            r_parts = pmain.tile([P, 2 * TI], f32, name="r_parts", tag="r_parts")
            dvec_raw = pmain.tile([P, TI], f32, name="dvec_raw", tag="dvec_raw")
            dvec = pmain.tile([P, TI], f32, name="dvec", tag="dvec")
            negd = pmain.tile([P, TI], f32, name="negd", tag="negd")
            inv_a_part = pmain.tile([P, TI], f32, name="inv_a_part", tag="inv_a_part")
            lsa = pmain.tile([P, TI], f32, name="lsa", tag="lsa")
            if relu_split:
                zeros_t = pmain.tile([P, HALF], bf16, name="zeros_t", tag="zeros_t")
                nc.vector.memset(zeros_t, 0.0)

            nc.vector.memset(ones_t, 1.0)
            nc.sync.dma_start(out=ident_sb, in_=ident_dram[:, :])

            # ---- loads: HBM [N, 128] column slabs -> SBUF [128, N] via xbar.
            # key (tx) first: phase A's moving operand, gates the pipeline.
            for c in range(KC):
                eng = nc.scalar if (split_loads and c % 2 == 0) else nc.sync
                eng.dma_start(out=kT[c], in_=tx[:, c * P:(c + 1) * P], transpose=True)
            for c in range(KC):
                eng = nc.scalar if (split_loads and c % 2) else nc.sync
                eng.dma_start(out=rT[c], in_=pt[:, c * P:(c + 1) * P], transpose=True)

            with tc.tile_pool(name="scr", bufs=2) as pscr:
                # ---- normalize in transposed layout.  kT is normalized
                # in place (phase A moving + phase B lhsT).  rT stays raw
                # (phase A lhsT; its row norm is applied via the ACT scale);
                # the normalized copy rTn is phase B's moving operand and is
                # produced concurrently with phase A's matmuls.
                with tc.tile_pool(name="psnorm", bufs=1, space="PSUM") as ppn:
                    for name, xT, apply_to in (("k", kT, None), ("r", rT, rTn)):
                        ssq_ps = ppn.tile([P, N], f32, name=f"ssq_{name}", tag="ssq")
                        for c in range(KC):
                            sq = pscr.tile([P, N], bf16, name=f"sq_{name}{c}", tag="sq")
                            nc.vector.tensor_mul(sq, xT[c], xT[c])
                            # partition-reduce with broadcast: out[m, j] = sum_d sq[d, j]
                            for jb in range(N // 512):
                                nc.tensor.matmul(
                                    ssq_ps[:, jb * 512:(jb + 1) * 512],
                                    lhsT=ones_t,
                                    rhs=sq[:, jb * 512:(jb + 1) * 512],
                                    start=(c == 0),
                                    stop=(c == KC - 1),
                                )
                        inv_n = pscr.tile([P, N], f32, name=f"inv_{name}", tag="inv")
                        nc.scalar.activation(inv_n, ssq_ps, Act.Sqrt)
                        nc.vector.reciprocal(inv_n, inv_n)
                        if apply_to is None:
                            for c in range(KC):
                                nc.vector.tensor_mul(xT[c], xT[c], inv_n)
                        else:
                            for c in range(KC):
                                nc.vector.tensor_mul(apply_to[c], xT[c], inv_n)
                            # per-partition row norms for the chunk rows:
                            # inv_a_part[p, ti] = inv_n[p, ti*128+p]
                            for ti in range(TI):
                                isc = pscr.tile([P, P], f32, name=f"isc_{ti}", tag="dsc")
                                nc.vector.tensor_mul(
                                    isc, inv_n[:, ti * P:(ti + 1) * P], ident_sb
                                )
                                nc.vector.tensor_reduce(
                                    inv_a_part[:, ti: ti + 1], isc,
                                    mybir.AxisListType.X, Alu.add,
                                )
                            nc.vector.tensor_scalar_mul(
                                lsa, inv_a_part, float(logit_scale)
                            )

                # ---- main phases
                with tc.tile_pool(name="psmm", bufs=1, space="PSUM") as ppm:
                    # phase A: sim rows chunk.  lhsT = rT[:, ti*128:...],
                    # rhs = kT (all N columns).  rowsum(exp) + diag extract.
                    for ti in range(TI):
                        for jh in range(2):
                            ps = ppm.tile([P, HALF], f32, name=f"psA_{ti}_{jh}", tag="mm", bufs=2)
                            for j4 in range(JB4):
                                jb = jh * JB4 + j4
                                for c in range(KC):
                                    nc.tensor.matmul(
                                        ps[:, j4 * 512:(j4 + 1) * 512],
                                        lhsT=rT[c][:, ti * P:(ti + 1) * P],
                                        rhs=kT[c][:, jb * 512:(jb + 1) * 512],
                                        start=(c == 0),
                                        stop=(c == KC - 1),
                                    )
                            esc = pscr.tile([P, HALF], bf16, name=f"escA_{ti}_{jh}", tag="esc")
                            nc.scalar.activation(
                                esc, ps, Act.Exp,
                                scale=lsa[:, ti: ti + 1],
                                accum_out=sp_parts[:, 2 * ti + jh: 2 * ti + jh + 1],
                            )
                            if jh == 0:
                                dsc = pscr.tile([P, P], f32, name=f"dsc_{ti}", tag="dsc")
                                nc.vector.tensor_mul(
                                    dsc, ps[:, ti * P:(ti + 1) * P], ident_sb
                                )
                                nc.vector.tensor_reduce(
                                    dvec_raw[:, ti: ti + 1], dsc,
                                    mybir.AxisListType.X, Alu.add,
                                )

                    # phase B: simT rows chunk. lhsT = kT[:, ti*128:...],
                    # rhs = rTn (all N).  rowsum(exp) + relu-rank rowsums.
                    # d = raw diag * inv_a (phase A's psum lacked the row norm)
                    nc.vector.tensor_mul(dvec, dvec_raw, inv_a_part)
                    nc.vector.tensor_scalar_mul(negd, dvec, -1.0)
                    for ti in range(TI):
                        for jh in range(2):
                            ps = ppm.tile([P, HALF], f32, name=f"psB_{ti}_{jh}", tag="mm", bufs=2)
                            for j4 in range(JB4):
                                ib = jh * JB4 + j4
                                for c in range(KC):
                                    nc.tensor.matmul(
                                        ps[:, j4 * 512:(j4 + 1) * 512],
                                        lhsT=kT[c][:, ti * P:(ti + 1) * P],
                                        rhs=rTn[c][:, ib * 512:(ib + 1) * 512],
                                        start=(c == 0),
                                        stop=(c == KC - 1),
                                    )
                            esc = pscr.tile([P, HALF], bf16, name=f"escB_{ti}_{jh}", tag="esc")
                            nc.scalar.activation(
                                esc, ps, Act.Exp,
                                scale=float(logit_scale),
                                accum_out=st_parts[:, 2 * ti + jh: 2 * ti + jh + 1],
                            )
                            rsc = pscr.tile([P, HALF], bf16, name=f"rscB_{ti}_{jh}", tag="rsc")
                            if relu_split and ti % 2 == 1:
                                nc.vector.scalar_tensor_tensor(
                                    out=rsc, in0=ps,
                                    scalar=dvec[:, ti: ti + 1],
                                    in1=zeros_t,
                                    op0=Alu.subtract, op1=Alu.max,
                                )
                                nc.vector.tensor_reduce(
                                    r_parts[:, 2 * ti + jh: 2 * ti + jh + 1], rsc,
                                    mybir.AxisListType.X, Alu.add,
                                )
                            else:
                                nc.scalar.activation(
                                    rsc, ps, Act.Relu,
                                    bias=negd[:, ti: ti + 1],
                                    accum_out=r_parts[:, 2 * ti + jh: 2 * ti + jh + 1],
                                )

            nc.sync.dma_start(out=out_sp[:, :], in_=sp_parts)
            nc.sync.dma_start(out=out_st[:, :], in_=st_parts)
            nc.sync.dma_start(out=out_r[:, :], in_=r_parts)
            nc.sync.dma_start(out=out_d[:, :], in_=dvec)

    _split_drain_waits(nc)
    return nc


def _rows_from_parts(parts: np.ndarray) -> np.ndarray:
    """[128, 2*TI] half-sums -> [CHUNK] in local row order."""
    s = parts[:, 0::2].astype(np.float64) + parts[:, 1::2].astype(np.float64)
    return s.T.reshape(-1)  # [ti, p] -> local row ti*128+p


def kernel(emb_point, emb_text, dist_norm, pos_idx, logit_scale):
    global LAST_RESULT
    import os

    ls = float(np.asarray(logit_scale, dtype=np.float64).reshape(-1)[0])
    nc = build_program(ls)

    in_maps = []
    for core in range(8):
        b, q = core // QPER, core % QPER
        refer = np.asarray(emb_point[b])[np.asarray(pos_idx[b])]
        key = np.asarray(emb_text[b])
        c0 = q * CHUNK
        in_maps.append({
            "pt": np.roll(refer, -c0, axis=0).astype(ml_dtypes.bfloat16),
            "tx": np.roll(key, -c0, axis=0).astype(ml_dtypes.bfloat16),
        })

    trace = bool(int(os.environ.get("KERNEL_TRACE", "0")))
    res = run_bass_kernel_spmd(nc, in_maps, list(range(8)), trace=trace)
    LAST_RESULT = res

    losses, ranks = [], []
    for b in range(B):
        sp = np.empty(N, np.float64)
        st = np.empty(N, np.float64)
        dd = np.empty(N, np.float64)
        rr = np.empty(N, np.float64)
        for q in range(QPER):
            r = res.results[b * QPER + q]
            sl = slice(q * CHUNK, (q + 1) * CHUNK)
            sp[sl] = _rows_from_parts(r["out_sp"])
            st[sl] = _rows_from_parts(r["out_st"])
            rr[sl] = _rows_from_parts(r["out_r"])
            dd[sl] = r["out_d"].astype(np.float64).T.reshape(-1)
        ce_p = np.log(sp) - ls * dd
        ce_t = np.log(st) - ls * dd
        dn = np.asarray(dist_norm[b], dtype=np.float64)
        losses.append(np.mean(0.5 * (ce_p + ce_t) * dn))
        ranks.append(np.sum(rr))

    contrastive = np.float32(np.mean(losses))
    rank_loss = np.float32(0.5 * np.mean(ranks))
    return contrastive, rank_loss



# revision 4
# speedup vs baseline: 2.6452x; 2.6452x over previous
"""Trainium2 Bass kernel for nn_ContrastiveCriterion.

Reference semantics (per sample b of B=2, N=4096, D=512):
    refer = l2_normalize(emb_point[b][pos_idx[b]])      # [N, D]
    key   = l2_normalize(emb_text[b])                   # [N, D]
    sim   = refer @ key.T                               # [N, N]
    ce_p[i] = logsumexp_j(ls*sim[i,j]) - ls*sim[i,i]
    ce_t[j] = logsumexp_i(ls*sim[i,j]) - ls*sim[j,j]
    loss_b  = mean_i(0.5*(ce_p+ce_t)*dist_norm[b])
    rank_b  = sum_ij relu(sim[i,j] - sim[j,j])
    out = (mean_b loss_b, 0.5 * mean_b rank_b)

Design: 8 cores = 2 samples x 4 row-chunks of 1024.  The host gathers,
l2-normalizes, computes the diagonal d[j] = refer_n[j]@key_n[j], and ships
pre-transposed operands, so the device makes a SINGLE pass over the sim
matrix in TRANSPOSED layout T[j, i] = sim[i, j] (j on partitions, the
core's 1024 rows i on the free axis).  Per j-tile [128, 1024]:
  - exp rowsums via ACT Exp accum_out      -> st partials (ce_t side)
  - rank sums via one DVE tensor_scalar    -> (T - d_j) relu'd + accum,
    d_j is a per-partition scalar in this layout (why we transpose)
  - exp colsums via ones-matmul into a persistent PSUM accumulator
    across all 32 j-tiles                  -> sp = full rowsums (ce_p side)
Host does the final O(N) f64 reductions (log, dn-weighting, means).
"""

import numpy as np
import ml_dtypes

import concourse.bass as bass
import concourse.tile as tile
import concourse.mybir as mybir
from concourse.bass_utils import run_bass_kernel_spmd

B, N, D = 2, 4096, 512
P = 128                 # SBUF partitions
KC = D // P             # 4 contraction chunks
QPER = 4                # cores per sample
CHUNK = N // QPER       # 1024 rows per core
JT = N // P             # 32 j-tiles per core
JQ = 4                  # kT load quarters
JPQ = JT // JQ          # 8 j-tiles per quarter
IB = CHUNK // 512       # 2 matmul free blocks per tile

bf16 = mybir.dt.bfloat16
f32 = mybir.dt.float32

# set by kernel() for test harness introspection
LAST_RESULT = None

# walrus codegen for TRN2 CTRL instructions (Drain) accepts a limited number
# of sync-wait slots; Tile's kernel-tail drain can carry one wait per live
# semaphore.  Split any over-limit drain into a chain of drains, each
# carrying at most MAX_DRAIN_WAITS waits (same-engine program order makes
# the chain equivalent to the single multi-wait drain).
MAX_DRAIN_WAITS = 1


def _split_drain_waits(nc: bass.Bass, max_waits: int = MAX_DRAIN_WAITS) -> None:
    for fn in nc.m.functions:
        for bb in fn.blocks:
            insts = list(bb.instructions)
            out, n_extra = [], 0
            for ins in insts:
                si = ins.sync_info
                if si is not None and si.on_wait and len(si.on_wait) > max_waits:
                    waits = list(si.on_wait)
                    for k in range(0, len(waits) - max_waits, max_waits):
                        extra = mybir.InstDrain(
                            name=f"{ins.name}_prewait{k}",
                            ins=[],
                            outs=[],
                        )
                        extra.engine = ins.engine
                        extra.sync_info = mybir.SyncInfo(
                            on_wait=waits[k: k + max_waits], on_update=[]
                        )
                        out.append(extra)
                        n_extra += 1
                    si.on_wait = waits[len(waits) - max_waits:]
                out.append(ins)
            if n_extra:
                bb.instructions[:] = out


def build_program(logit_scale: float, fp8: bool = False,
                  esc_fp8: bool = False) -> bass.Bass:
    nc = bass.Bass()

    mm_dt = mybir.dt.float8e4 if fp8 else bf16
    esc_dt = mybir.dt.float8e4 if esc_fp8 else bf16

    ptT = nc.declare_dram_parameter("ptT", [D, CHUNK], mm_dt, isOutput=False)
    txT = nc.declare_dram_parameter("txT", [D, N], mm_dt, isOutput=False)
    db = nc.declare_dram_parameter("db", [P, JT], f32, isOutput=False)
    out_st = nc.declare_dram_parameter("out_st", [P, JT], f32, isOutput=True)
    out_r = nc.declare_dram_parameter("out_r", [P, JT], f32, isOutput=True)
    out_sp = nc.declare_dram_parameter("out_sp", [1, CHUNK], f32, isOutput=True)

    Act = mybir.ActivationFunctionType
    Alu = mybir.AluOpType
    ls = float(logit_scale)

    with tile.TileContext(nc) as tc:
        with tc.tile_pool(name="main", bufs=1) as pmain:
            # persistent transposed operands: kT[c][q][d_local, j], rT[c][d_local, i]
            kT = [[pmain.tile([P, CHUNK], mm_dt, name=f"kT{c}_{q}", tag=f"kT{c}_{q}")
                   for q in range(JQ)] for c in range(KC)]
            rT = [pmain.tile([P, CHUNK], mm_dt, name=f"rT{c}", tag=f"rT{c}")
                  for c in range(KC)]
            dparts = pmain.tile([P, JT], f32, name="dparts", tag="dparts")
            ones_t = pmain.tile([P, P], esc_dt, name="ones_t", tag="ones_t")
            st_parts = pmain.tile([P, JT], f32, name="st_parts", tag="st_parts")
            r_parts = pmain.tile([P, JT], f32, name="r_parts", tag="r_parts")
            sp_sb = pmain.tile([1, CHUNK], f32, name="sp_sb", tag="sp_sb")
            zeros_t = pmain.tile([P, CHUNK], bf16, name="zeros_t", tag="zeros_t")

            nc.vector.memset(ones_t, 1.0)
            nc.vector.memset(zeros_t, 0.0)

            # loads (straight DMA, host pre-transposed); two queues so the
            # first tile's operands (rT + kT quarter 0) land fast
            nc.gpsimd.dma_start(out=dparts, in_=db[:, :])
            for c in range(KC):
                eng = nc.sync if c % 2 == 0 else nc.gpsimd
                eng.dma_start(out=rT[c], in_=ptT[c * P:(c + 1) * P, :])
            for q in range(JQ):
                for c in range(KC):
                    eng = nc.sync if c % 2 == 0 else nc.gpsimd
                    eng.dma_start(
                        out=kT[c][q],
                        in_=txT[c * P:(c + 1) * P, q * CHUNK:(q + 1) * CHUNK],
                    )

            with tc.tile_pool(name="pscs", bufs=1, space="PSUM") as pcs:
                # persistent colsum-of-exp accumulator (full rowsums of sim
                # for the core's rows); ones-matmul broadcasts over partitions
                cs = pcs.tile([P, CHUNK], f32, name="cs", tag="cs")
                with tc.tile_pool(name="psmm", bufs=3, space="PSUM") as pmm, \
                        tc.tile_pool(name="scr", bufs=3) as pscr:
                    for jt in range(JT):
                        q, jl = jt // JPQ, jt % JPQ
                        ps = pmm.tile([P, CHUNK], f32, name=f"ps{jt}", tag="mm")
                        for c in range(KC):
                            for ib in range(IB):
                                nc.tensor.matmul(
                                    ps[:, ib * 512:(ib + 1) * 512],
                                    lhsT=kT[c][q][:, jl * P:(jl + 1) * P],
                                    rhs=rT[c][:, ib * 512:(ib + 1) * 512],
                                    start=(c == 0),
                                    stop=(c == KC - 1),
                                )
                        esc = pscr.tile([P, CHUNK], esc_dt, name=f"esc{jt}", tag="esc")
                        nc.scalar.activation(
                            esc, ps, Act.Exp, scale=ls,
                            accum_out=st_parts[:, jt: jt + 1],
                        )
                        rsc = pscr.tile([P, CHUNK], bf16, name=f"rsc{jt}", tag="rsc")
                        # out = max(ps - d_j, 0); accum_out = sum(out) (fixed add)
                        nc.vector.scalar_tensor_tensor(
                            out=rsc, in0=ps,
                            scalar=dparts[:, jt: jt + 1], in1=zeros_t,
                            op0=Alu.subtract, op1=Alu.max,
                            accum_out=r_parts[:, jt: jt + 1],
                        )
                        for ib in range(IB):
                            nc.tensor.matmul(
                                cs[:, ib * 512:(ib + 1) * 512],
                                lhsT=ones_t,
                                rhs=esc[:, ib * 512:(ib + 1) * 512],
                                start=(jt == 0),
                                stop=(jt == JT - 1),
                            )
                    # all partitions of cs hold the same colsums; row 0 is enough
                    nc.scalar.activation(sp_sb, cs[0:1, :], Act.Copy)

            nc.sync.dma_start(out=out_st[:, :], in_=st_parts)
            nc.sync.dma_start(out=out_r[:, :], in_=r_parts)
            nc.sync.dma_start(out=out_sp[:, :], in_=sp_sb)

    _split_drain_waits(nc)
    return nc


def kernel(emb_point, emb_text, dist_norm, pos_idx, logit_scale):
    global LAST_RESULT
    import os

    fp8 = bool(int(os.environ.get("KERNEL_FP8", "0")))
    esc_fp8 = bool(int(os.environ.get("KERNEL_ESC_FP8", "0")))
    cast_dt = ml_dtypes.float8_e4m3 if fp8 else ml_dtypes.bfloat16

    ls = float(np.asarray(logit_scale, dtype=np.float64).reshape(-1)[0])
    nc = build_program(ls, fp8=fp8, esc_fp8=esc_fp8)

    in_maps = []
    dvecs = []
    for b in range(B):
        ep = np.asarray(emb_point[b], dtype=np.float32)
        et = np.asarray(emb_text[b], dtype=np.float32)
        refer = ep[np.asarray(pos_idx[b])]
        rn = refer / np.maximum(
            np.linalg.norm(refer, axis=1, keepdims=True), 1e-12)
        kn = et / np.maximum(np.linalg.norm(et, axis=1, keepdims=True), 1e-12)
        d = np.einsum("nd,nd->n", rn.astype(np.float64), kn.astype(np.float64))
        dvecs.append(d)
        txT_b = np.ascontiguousarray(kn.T).astype(cast_dt)
        rnT = np.ascontiguousarray(rn.T).astype(cast_dt)
        db_b = np.ascontiguousarray(
            d.astype(np.float32).reshape(JT, P).T)
        for q in range(QPER):
            in_maps.append({
                "ptT": np.ascontiguousarray(rnT[:, q * CHUNK:(q + 1) * CHUNK]),
                "txT": txT_b,
                "db": db_b,
            })

    trace = bool(int(os.environ.get("KERNEL_TRACE", "0")))
    res = run_bass_kernel_spmd(nc, in_maps, list(range(8)), trace=trace)
    LAST_RESULT = res

    losses, ranks = [], []
    for b in range(B):
        d = dvecs[b]
        sp = np.empty(N, np.float64)
        st = np.zeros(N, np.float64)
        rank = 0.0
        for q in range(QPER):
            r = res.results[b * QPER + q]
            sp[q * CHUNK:(q + 1) * CHUNK] = (
                r["out_sp"].astype(np.float64).reshape(-1))
            # out_st[p, jt] is the partial colsum for j = jt*128 + p
            st += r["out_st"].astype(np.float64).T.reshape(-1)
            rank += float(r["out_r"].astype(np.float64).sum())
        ce_p = np.log(sp) - ls * d
        ce_t = np.log(st) - ls * d
        dn = np.asarray(dist_norm[b], dtype=np.float64)
        losses.append(np.mean(0.5 * (ce_p + ce_t) * dn))
        ranks.append(rank)

    contrastive = np.float32(np.mean(losses))
    rank_loss = np.float32(0.5 * np.mean(ranks))
    return contrastive, rank_loss


# revision 5
# speedup vs baseline: 3.7886x; 1.4323x over previous
"""Trainium2 Bass kernel for nn_ContrastiveCriterion.

Reference semantics (per sample b of B=2, N=4096, D=512):
    refer = l2_normalize(emb_point[b][pos_idx[b]])      # [N, D]
    key   = l2_normalize(emb_text[b])                   # [N, D]
    sim   = refer @ key.T                               # [N, N]
    ce_p[i] = logsumexp_j(ls*sim[i,j]) - ls*sim[i,i]
    ce_t[j] = logsumexp_i(ls*sim[i,j]) - ls*sim[j,j]
    loss_b  = mean_i(0.5*(ce_p+ce_t)*dist_norm[b])
    rank_b  = sum_ij relu(sim[i,j] - sim[j,j])
    out = (mean_b loss_b, 0.5 * mean_b rank_b)

Design: 8 cores = 2 samples x 4 row-chunks of 1024.  The host gathers,
l2-normalizes, computes the diagonal d[j] = refer_n[j]@key_n[j], and ships
pre-transposed operands, so the device makes a SINGLE pass over the sim
matrix in TRANSPOSED layout T[j, i] = sim[i, j] (j on partitions, the
core's 1024 rows i on the free axis).  Per j-tile [128, 1024]:
  - exp rowsums via ACT Exp accum_out      -> st partials (ce_t side)
  - rank sums via one DVE tensor_scalar    -> (T - d_j) relu'd + accum,
    d_j is a per-partition scalar in this layout (why we transpose)
  - exp colsums via ones-matmul into a persistent PSUM accumulator
    across all 32 j-tiles                  -> sp = full rowsums (ce_p side)
Host does the final O(N) f64 reductions (log, dn-weighting, means).
"""

import numpy as np
import ml_dtypes

import concourse.bass as bass
import concourse.tile as tile
import concourse.mybir as mybir
from concourse.bass_utils import run_bass_kernel_spmd

B, N, D = 2, 4096, 512
P = 128                 # SBUF partitions
KC = D // P             # 4 contraction chunks
QPER = 4                # cores per sample
CHUNK = N // QPER       # 1024 rows per core
JT = N // P             # 32 j-tiles per core
JQ = 4                  # kT load quarters
JPQ = JT // JQ          # 8 j-tiles per quarter
IB = CHUNK // 512       # 2 matmul free blocks per tile

bf16 = mybir.dt.bfloat16
f32 = mybir.dt.float32

# set by kernel() for test harness introspection
LAST_RESULT = None

# walrus codegen for TRN2 CTRL instructions (Drain) accepts a limited number
# of sync-wait slots; Tile's kernel-tail drain can carry one wait per live
# semaphore.  Split any over-limit drain into a chain of drains, each
# carrying at most MAX_DRAIN_WAITS waits (same-engine program order makes
# the chain equivalent to the single multi-wait drain).
MAX_DRAIN_WAITS = 1


def _split_drain_waits(nc: bass.Bass, max_waits: int = MAX_DRAIN_WAITS) -> None:
    for fn in nc.m.functions:
        for bb in fn.blocks:
            insts = list(bb.instructions)
            out, n_extra = [], 0
            for ins in insts:
                si = ins.sync_info
                if si is not None and si.on_wait and len(si.on_wait) > max_waits:
                    waits = list(si.on_wait)
                    for k in range(0, len(waits) - max_waits, max_waits):
                        extra = mybir.InstDrain(
                            name=f"{ins.name}_prewait{k}",
                            ins=[],
                            outs=[],
                        )
                        extra.engine = ins.engine
                        extra.sync_info = mybir.SyncInfo(
                            on_wait=waits[k: k + max_waits], on_update=[]
                        )
                        out.append(extra)
                        n_extra += 1
                    si.on_wait = waits[len(waits) - max_waits:]
                out.append(ins)
            if n_extra:
                bb.instructions[:] = out


def build_program(logit_scale: float, fp8: bool = False,
                  esc_fp8: bool = False) -> bass.Bass:
    nc = bass.Bass()

    mm_dt = mybir.dt.float8e4 if fp8 else bf16
    esc_dt = mybir.dt.float8e4 if esc_fp8 else bf16

    ptT = nc.declare_dram_parameter("ptT", [D, CHUNK], mm_dt, isOutput=False)
    txT = nc.declare_dram_parameter("txT", [D, N], mm_dt, isOutput=False)
    db = nc.declare_dram_parameter("db", [P, JT], f32, isOutput=False)
    out_st = nc.declare_dram_parameter("out_st", [P, JT], f32, isOutput=True)
    out_r = nc.declare_dram_parameter("out_r", [P, JT], f32, isOutput=True)
    out_sp = nc.declare_dram_parameter("out_sp", [1, CHUNK], f32, isOutput=True)

    Act = mybir.ActivationFunctionType
    Alu = mybir.AluOpType
    ls = float(logit_scale)

    Dr = mybir.MatmulPerfMode.DoubleRow

    with tile.TileContext(nc) as tc:
        with tc.tile_pool(name="main", bufs=1) as pmain:
            # persistent transposed operands.  bf16: kT[c][q] [P, CHUNK],
            # rT[c] [P, CHUNK] (c = 4 contraction chunks of 128).  fp8
            # DoubleRow: chunks paired along a 2-slot free dim so each
            # matmul contracts 256 rows at 0.5 cycles/row.
            if fp8:
                kT = [[pmain.tile([P, 2, CHUNK], mm_dt, name=f"kT{cp}_{q}",
                                  tag=f"kT{cp}_{q}") for q in range(JQ)]
                      for cp in range(KC // 2)]
                rT = [pmain.tile([P, 2, CHUNK], mm_dt, name=f"rT{cp}",
                                 tag=f"rT{cp}") for cp in range(KC // 2)]
            else:
                kT = [[pmain.tile([P, CHUNK], mm_dt, name=f"kT{c}_{q}",
                                  tag=f"kT{c}_{q}") for q in range(JQ)]
                      for c in range(KC)]
                rT = [pmain.tile([P, CHUNK], mm_dt, name=f"rT{c}", tag=f"rT{c}")
                      for c in range(KC)]
            dparts = pmain.tile([P, JT], f32, name="dparts", tag="dparts")
            if esc_fp8:
                ones_t = pmain.tile([P, 2, P], esc_dt, name="ones_t", tag="ones_t")
            else:
                ones_t = pmain.tile([P, P], esc_dt, name="ones_t", tag="ones_t")
            st_parts = pmain.tile([P, JT], f32, name="st_parts", tag="st_parts")
            r_parts = pmain.tile([P, JT], f32, name="r_parts", tag="r_parts")
            sp_sb = pmain.tile([1, CHUNK], f32, name="sp_sb", tag="sp_sb")
            zeros_t = pmain.tile([P, CHUNK], bf16, name="zeros_t", tag="zeros_t")

            nc.vector.memset(ones_t, 1.0)
            nc.vector.memset(zeros_t, 0.0)

            # loads (straight DMA, host pre-transposed); two queues so the
            # first tile's operands (rT + kT quarter 0) land fast
            nc.gpsimd.dma_start(out=dparts, in_=db[:, :])
            for c in range(KC):
                eng = nc.sync if c % 2 == 0 else nc.gpsimd
                dst = rT[c // 2][:, c % 2: c % 2 + 1, :] if fp8 else rT[c]
                eng.dma_start(out=dst, in_=ptT[c * P:(c + 1) * P, :])
            for q in range(JQ):
                for c in range(KC):
                    eng = nc.sync if c % 2 == 0 else nc.gpsimd
                    dst = (kT[c // 2][q][:, c % 2: c % 2 + 1, :] if fp8
                           else kT[c][q])
                    eng.dma_start(
                        out=dst,
                        in_=txT[c * P:(c + 1) * P, q * CHUNK:(q + 1) * CHUNK],
                    )

            with tc.tile_pool(name="pscs", bufs=1, space="PSUM") as pcs:
                # persistent colsum-of-exp accumulator (full rowsums of sim
                # for the core's rows); ones-matmul broadcasts over partitions
                cs = pcs.tile([P, CHUNK], f32, name="cs", tag="cs")
                with tc.tile_pool(name="psmm", bufs=3, space="PSUM") as pmm, \
                        tc.tile_pool(name="scr", bufs=3) as pscr:
                    escd = None
                    for jt in range(JT):
                        q, jl = jt // JPQ, jt % JPQ
                        ps = pmm.tile([P, CHUNK], f32, name=f"ps{jt}", tag="mm")
                        if fp8:
                            for cp in range(KC // 2):
                                for ib in range(IB):
                                    nc.tensor.matmul(
                                        ps[:, ib * 512:(ib + 1) * 512],
                                        lhsT=kT[cp][q][:, :, jl * P:(jl + 1) * P],
                                        rhs=rT[cp][:, :, ib * 512:(ib + 1) * 512],
                                        start=(cp == 0),
                                        stop=(cp == KC // 2 - 1),
                                        perf_mode=Dr,
                                    )
                        else:
                            for c in range(KC):
                                for ib in range(IB):
                                    nc.tensor.matmul(
                                        ps[:, ib * 512:(ib + 1) * 512],
                                        lhsT=kT[c][q][:, jl * P:(jl + 1) * P],
                                        rhs=rT[c][:, ib * 512:(ib + 1) * 512],
                                        start=(c == 0),
                                        stop=(c == KC - 1),
                                    )
                        if esc_fp8:
                            if jt % 2 == 0:
                                escd = pscr.tile([P, 2, CHUNK], esc_dt,
                                                 name=f"esc{jt}", tag="esc")
                            esc_out = escd[:, jt % 2: jt % 2 + 1, :]
                        else:
                            escd = pscr.tile([P, CHUNK], esc_dt,
                                             name=f"esc{jt}", tag="esc")
                            esc_out = escd
                        nc.scalar.activation(
                            esc_out, ps, Act.Exp, scale=ls,
                            accum_out=st_parts[:, jt: jt + 1],
                        )
                        rsc = pscr.tile([P, CHUNK], bf16, name=f"rsc{jt}", tag="rsc")
                        # out = max(ps - d_j, 0); accum_out = sum(out) (fixed add)
                        nc.vector.scalar_tensor_tensor(
                            out=rsc, in0=ps,
                            scalar=dparts[:, jt: jt + 1], in1=zeros_t,
                            op0=Alu.subtract, op1=Alu.max,
                            accum_out=r_parts[:, jt: jt + 1],
                        )
                        if esc_fp8:
                            if jt % 2 == 1:
                                for ib in range(IB):
                                    nc.tensor.matmul(
                                        cs[:, ib * 512:(ib + 1) * 512],
                                        lhsT=ones_t,
                                        rhs=escd[:, :, ib * 512:(ib + 1) * 512],
                                        start=(jt == 1),
                                        stop=(jt == JT - 1),
                                        perf_mode=Dr,
                                    )
                        else:
                            for ib in range(IB):
                                nc.tensor.matmul(
                                    cs[:, ib * 512:(ib + 1) * 512],
                                    lhsT=ones_t,
                                    rhs=escd[:, ib * 512:(ib + 1) * 512],
                                    start=(jt == 0),
                                    stop=(jt == JT - 1),
                                )
                    # all partitions of cs hold the same colsums; row 0 is enough
                    nc.scalar.activation(sp_sb, cs[0:1, :], Act.Copy)

            nc.sync.dma_start(out=out_st[:, :], in_=st_parts)
            nc.sync.dma_start(out=out_r[:, :], in_=r_parts)
            nc.sync.dma_start(out=out_sp[:, :], in_=sp_sb)

    _split_drain_waits(nc)
    return nc


def kernel(emb_point, emb_text, dist_norm, pos_idx, logit_scale):
    global LAST_RESULT
    import os

    fp8 = bool(int(os.environ.get("KERNEL_FP8", "0")))
    esc_fp8 = bool(int(os.environ.get("KERNEL_ESC_FP8", "0")))
    cast_dt = ml_dtypes.float8_e4m3 if fp8 else ml_dtypes.bfloat16

    ls = float(np.asarray(logit_scale, dtype=np.float64).reshape(-1)[0])
    nc = build_program(ls, fp8=fp8, esc_fp8=esc_fp8)

    in_maps = []
    dvecs = []
    for b in range(B):
        ep = np.asarray(emb_point[b], dtype=np.float32)
        et = np.asarray(emb_text[b], dtype=np.float32)
        refer = ep[np.asarray(pos_idx[b])]
        rn = refer / np.maximum(
            np.linalg.norm(refer, axis=1, keepdims=True), 1e-12)
        kn = et / np.maximum(np.linalg.norm(et, axis=1, keepdims=True), 1e-12)
        d = np.einsum("nd,nd->n", rn.astype(np.float64), kn.astype(np.float64))
        dvecs.append(d)
        txT_b = np.ascontiguousarray(kn.T).astype(cast_dt)
        rnT = np.ascontiguousarray(rn.T).astype(cast_dt)
        db_b = np.ascontiguousarray(
            d.astype(np.float32).reshape(JT, P).T)
        for q in range(QPER):
            in_maps.append({
                "ptT": np.ascontiguousarray(rnT[:, q * CHUNK:(q + 1) * CHUNK]),
                "txT": txT_b,
                "db": db_b,
            })

    trace = bool(int(os.environ.get("KERNEL_TRACE", "0")))
    res = run_bass_kernel_spmd(nc, in_maps, list(range(8)), trace=trace)
    LAST_RESULT = res

    losses, ranks = [], []
    for b in range(B):
        d = dvecs[b]
        sp = np.empty(N, np.float64)
        st = np.zeros(N, np.float64)
        rank = 0.0
        for q in range(QPER):
            r = res.results[b * QPER + q]
            sp[q * CHUNK:(q + 1) * CHUNK] = (
                r["out_sp"].astype(np.float64).reshape(-1))
            # out_st[p, jt] is the partial colsum for j = jt*128 + p
            st += r["out_st"].astype(np.float64).T.reshape(-1)
            rank += float(r["out_r"].astype(np.float64).sum())
        ce_p = np.log(sp) - ls * d
        ce_t = np.log(st) - ls * d
        dn = np.asarray(dist_norm[b], dtype=np.float64)
        losses.append(np.mean(0.5 * (ce_p + ce_t) * dn))
        ranks.append(rank)

    contrastive = np.float32(np.mean(losses))
    rank_loss = np.float32(0.5 * np.mean(ranks))
    return contrastive, rank_loss
